# revision 11
# baseline (speedup 1.0000x reference)
"""Trainium2 Bass kernel: batched Ising energies E_b = s_b^T J s_b.

state: [1024, 2048] float32 in {0,1};  J: [2048, 2048] float32.
Returns energies [1024] float32.

Because s_i^2 = 1, E = s^T J s = s^T A s where A folds the symmetric
part of J into the upper block-triangle (A_ij = J_ij + J_ji for i<j,
A_ii = J_ii, zero below).  That halves both the matmul work and the
J bytes moved; A ships as bf16 (rel err ~2.5e-3, tolerance 2e-2).

Sharding (8 cores): 2 batch halves x 4 column groups.  Core (r, c)
owns column tiles {4j+r : j=0..3} of A for batch half c.  For a single
SPMD program across cores with different triangle supports, slot j
accumulates its ctile over a fixed cap of 4(j+1) contraction tiles
(ragged edge zero-padded on the host) and ktile storage order is
permuted per core so ctile 4j+r's spin rows always sit at position
4j+3 (uniform epilogue addressing).

Per core: A tiles are the stationary operand; spins ship directly as
fp8e4 {-1,+1} (exact, 1 byte) and stream as the moving operand -- the
PE accepts bf16 stationary x fp8 moving, so no on-chip expand at all.
psum[j] = g = A_block^T s.  Epilogue per slot: m = psum[j] * spin rows
(DVE, fp8 operand), then a ones-vector matmul reduces m across
partitions into a [1, 512] psum accumulator; one output DMA.  Input
streams ride both HWDGE rings (spins on sync, A on scalar) in exact
consumption order with small head chunks; dummy matmuls warm the PE
clock gate while the first chunks land.
"""

import sys

if "/opt/trn_rl_repo" not in sys.path:
    sys.path.insert(0, "/opt/trn_rl_repo")

import numpy as np
import ml_dtypes

B, N = 1024, 2048
P = 128
KT = N // P          # 16 contraction/column tiles
R, C = 4, 2          # column groups x batch halves
BH = B // C          # 512 samples per core
S_CAP = [4, 8, 12, 16]   # per-slot ktile caps (uniform across cores)
ORDER = [(s, j) for s in range(KT) for j in range(4) if S_CAP[j] > s]
NTILE = len(ORDER)   # 40 stationary tiles per core
N_WARM = 70          # PE clock-gate warmup matmuls
WARM_N = 64

# chunk boundaries (in ktile positions) for the two input streams
SU_CHUNKS = [(0, 2), (2, 8), (8, 16)]
A_CHUNKS = [(0, 2), (2, 8), (8, 16)]

_cache = {}


def _tiles_before(s):
    return sum(1 for (s_, _) in ORDER if s_ < s)


def _build_program():
    import concourse.bacc as bacc
    import concourse.mybir as mybir
    import concourse.tile as tile

    bf16 = mybir.dt.bfloat16
    f32 = mybir.dt.float32
    fp8 = mybir.dt.float8e4

    nc = bacc.Bacc("TRN2", target_bir_lowering=False, debug=False, num_devices=R * C)

    su_ext = nc.dram_tensor("su", [P, KT * BH], fp8, kind="ExternalInput").ap()
    a_ext = nc.dram_tensor("a", [P, NTILE * P], bf16, kind="ExternalInput").ap()
    out_ext = nc.dram_tensor("part", [1, BH], f32, kind="ExternalOutput").ap()

    with tile.TileContext(nc) as tc:
        with (
            tc.tile_pool(name="persist", bufs=1) as persist,
            tc.tile_pool(name="work", bufs=1) as work,
            tc.tile_pool(name="psum", bufs=1, space="PSUM") as psum_pool,
            tc.tile_pool(name="warmps", bufs=1, space="PSUM") as warm_pool,
        ):
            su_t = persist.tile([P, KT, BH], fp8)
            a_t = persist.tile([P, NTILE, P], bf16)
            ones_t = persist.tile([P, 1], bf16)
            warm_src = persist.tile([P, P], bf16)
            red_sb = persist.tile([1, BH], f32)
            m_t = [work.tile([P, BH], bf16, name=f"m_{j}") for j in range(4)]

            ps = [psum_pool.tile([P, BH], f32, name=f"ps_{j}") for j in range(4)]
            ep = psum_pool.tile([1, BH], f32, name="ep")
            warm_ps = warm_pool.tile([P, WARM_N], f32)

            # constants via gpsimd (released early in the preamble)
            nc.gpsimd.memset(warm_src[:], 0.0)
            nc.gpsimd.memset(ones_t[:], 1.0)

            # PE warmup: dummy matmuls keep the HAM activity window busy
            # while the first input chunks land.
            for _ in range(N_WARM):
                nc.tensor.matmul(
                    warm_ps, lhsT=warm_src[:, :P], rhs=warm_src[:, :WARM_N],
                    start=True, stop=True,
                )

            # input streams: spins on the sync HWDGE ring, A on the
            # scalar ring, in exact consumption order, small head chunks.
            su3 = su_ext.rearrange("p (k b) -> p k b", b=BH)
            a3 = a_ext.rearrange("p (t q) -> p t q", q=P)
            for ci in range(len(SU_CHUNKS)):
                k0, k1 = SU_CHUNKS[ci]
                nc.sync.dma_start(out=su_t[:, k0:k1], in_=su3[:, k0:k1])
                s0, s1 = A_CHUNKS[ci]
                t0, t1 = _tiles_before(s0), _tiles_before(s1)
                nc.gpsimd.dma_start(out=a_t[:, t0:t1], in_=a3[:, t0:t1])

            def epilogue(j):
                # m = ps[j] * spin rows of ctile j (position 4j+3), then
                # reduce across partitions via a ones-vector matmul.  The
                # last slot is split into sample halves so the DVE
                # multiply, ones-matmul and PSUM->SBUF copy pipeline.
                halves = (
                    [(0, BH)] if j < 3 else [(0, BH // 2), (BH // 2, BH)]
                )
                for (b0, b1) in halves:
                    nc.vector.scalar_tensor_tensor(
                        m_t[j][:, b0:b1],
                        ps[j][:, b0:b1],
                        1.0,
                        su_t[:, 4 * j + 3, b0:b1],
                        mybir.AluOpType.mult,
                        mybir.AluOpType.mult,
                    )
                    nc.tensor.matmul(
                        ep[:, b0:b1],
                        lhsT=ones_t[:],
                        rhs=m_t[j][:, b0:b1],
                        start=(j == 0),
                        stop=(j == 3),
                    )
                    if j == 3:
                        # PSUM -> SBUF per half (DMA cannot read PSUM)
                        nc.vector.tensor_scalar(
                            red_sb[:, b0:b1], ep[:, b0:b1], 1.0, 0.0,
                            mybir.AluOpType.mult, mybir.AluOpType.add,
                        )

            ti = 0
            for s in range(KT):
                for j in range(4):
                    if S_CAP[j] <= s:
                        continue
                    nc.tensor.matmul(
                        ps[j],
                        lhsT=a_t[:, ti],
                        rhs=su_t[:, s],
                        start=(s == 0),
                        stop=(s == S_CAP[j] - 1),
                    )
                    ti += 1
                for j in range(4):
                    if s == S_CAP[j] - 1:
                        epilogue(j)
            assert ti == NTILE

            nc.sync.dma_start(out=out_ext, in_=red_sb[:])

    nc.compile()
    return nc


def _make_in_maps(state, J):
    bf16 = ml_dtypes.bfloat16
    fp8 = ml_dtypes.float8_e4m3
    state = np.asarray(state, dtype=np.float32)
    J = np.asarray(J, dtype=np.float32)

    # fold the symmetric part into the upper block-triangle
    A = np.triu(J + J.T, 1) + np.diag(np.diag(J))
    A = A.astype(bf16)
    sp8 = np.where(state > 0.5, np.float32(1.0), np.float32(-1.0)).astype(fp8)

    in_maps = []
    placement = []
    for core in range(R * C):
        r, c = divmod(core, C)
        kt_of_pos = []
        for g in range(4):
            grp = [x for x in range(4 * g, 4 * g + 4) if x != 4 * g + r]
            kt_of_pos += grp + [4 * g + r]
        ctile = [4 * j + r for j in range(4)]

        # su: [P, KT, BH]; partition p holds, for position k, the
        # samples of spin row (kt_of_pos[k]*P + p)
        sm = sp8[c * BH:(c + 1) * BH]               # [BH, N]
        su = np.empty((P, KT, BH), dtype=fp8)
        for pos in range(KT):
            kt = kt_of_pos[pos]
            su[:, pos, :] = sm[:, kt * P:(kt + 1) * P].T
        # A tiles in consumption order; zero when above the triangle
        at = np.zeros((P, NTILE, P), dtype=bf16)
        for idx, (s, j) in enumerate(ORDER):
            kt, cj = kt_of_pos[s], ctile[j]
            if kt <= cj:
                at[:, idx, :] = A[kt * P:(kt + 1) * P, cj * P:(cj + 1) * P]
        in_maps.append({
            "su": np.ascontiguousarray(su.reshape(P, KT * BH)),
            "a": np.ascontiguousarray(at.reshape(P, NTILE * P)),
        })
        placement.append((r, c))
    return in_maps, placement


def kernel(state, J):
    from concourse.bass_utils import run_bass_kernel_spmd

    if "nc" not in _cache:
        _cache["nc"] = _build_program()
    nc = _cache["nc"]

    in_maps, placement = _make_in_maps(state, J)
    res = run_bass_kernel_spmd(nc, in_maps, list(range(R * C)))

    out = np.zeros(B, dtype=np.float32)
    for core, (r, c) in enumerate(placement):
        out[c * BH:(c + 1) * BH] += res.results[core]["part"].reshape(BH)
    return out


# revision 13
# speedup vs baseline: 1.1202x; 1.1202x over previous
"""Trainium2 Bass kernel: batched Ising energies E_b = s_b^T J s_b.

state: [1024, 2048] float32 in {0,1};  J: [2048, 2048] float32.
Returns energies [1024] float32.

Because s_i^2 = 1, E = s^T J s = s^T A s where A folds the symmetric
part of J into the upper block-triangle (A_ij = J_ij + J_ji for i<j,
A_ii = J_ii, zero below).  That halves both the matmul work and the
J bytes moved; A ships as bf16 (rel err ~2.5e-3, tolerance 2e-2).

Sharding (8 cores): 2 batch halves x 4 column groups.  Core (r, c)
owns column tiles {4j+r : j=0..3} of A for batch half c.  For a single
SPMD program across cores with different triangle supports, slot j
accumulates its ctile over a fixed cap of 4(j+1) contraction tiles
(ragged edge zero-padded on the host) and ktile storage order is
permuted per core so ctile 4j+r's spin rows always sit at position
4j+3 (uniform epilogue addressing).

Per core: A tiles are the stationary operand; spins ship directly as
fp8e4 {-1,+1} (exact, 1 byte) and stream as the moving operand -- the
PE accepts bf16 stationary x fp8 moving, so no on-chip expand at all.
psum[j] = g = A_block^T s.  Epilogue per slot: m = psum[j] * spin rows
(DVE, fp8 operand), then a ones-vector matmul reduces m across
partitions into a [1, 512] psum accumulator; one output DMA.  Input
streams ride both HWDGE rings (spins on sync, A on scalar) in exact
consumption order with small head chunks; dummy matmuls warm the PE
clock gate while the first chunks land.
"""

import sys

if "/opt/trn_rl_repo" not in sys.path:
    sys.path.insert(0, "/opt/trn_rl_repo")

import numpy as np
import ml_dtypes

B, N = 1024, 2048
P = 128
KT = N // P          # 16 contraction/column tiles
R, C = 4, 2          # column groups x batch halves
BH = B // C          # 512 samples per core
S_CAP = [4, 8, 12, 16]   # per-slot ktile caps (uniform across cores)
ORDER = [(s, j) for s in range(KT) for j in range(4) if S_CAP[j] > s]
NTILE = len(ORDER)   # 40 stationary tiles per core
N_WARM = 70          # PE clock-gate warmup matmuls
WARM_N = 64

# chunk boundaries (in ktile positions) for the two input streams
SU_CHUNKS = [(0, 2), (2, 6), (6, 16)]
A_CHUNKS = [(0, 2), (2, 6), (6, 16)]

_cache = {}


def _tiles_before(s):
    return sum(1 for (s_, _) in ORDER if s_ < s)


def _build_program():
    import concourse.bacc as bacc
    import concourse.mybir as mybir
    import concourse.tile as tile

    bf16 = mybir.dt.bfloat16
    f32 = mybir.dt.float32
    fp8 = mybir.dt.float8e4

    nc = bacc.Bacc("TRN2", target_bir_lowering=False, debug=False, num_devices=R * C)

    su_ext = nc.dram_tensor("su", [P, KT * BH], fp8, kind="ExternalInput").ap()
    a_ext = nc.dram_tensor("a", [P, NTILE * P], bf16, kind="ExternalInput").ap()
    out_ext = nc.dram_tensor("part", [1, BH], f32, kind="ExternalOutput").ap()

    with tile.TileContext(nc) as tc:
        with (
            tc.tile_pool(name="persist", bufs=1) as persist,
            tc.tile_pool(name="work", bufs=1) as work,
            tc.tile_pool(name="psum", bufs=1, space="PSUM") as psum_pool,
            tc.tile_pool(name="warmps", bufs=1, space="PSUM") as warm_pool,
        ):
            su_t = persist.tile([P, KT, BH], fp8)
            a_t = persist.tile([P, NTILE, P], bf16)
            ones_t = persist.tile([P, 1], bf16)
            warm_src = persist.tile([P, P], bf16)
            red_sb = persist.tile([1, BH], f32)
            m_t = [work.tile([P, BH], bf16, name=f"m_{j}") for j in range(4)]

            ps = [psum_pool.tile([P, BH], f32, name=f"ps_{j}") for j in range(4)]
            ep = psum_pool.tile([1, BH], f32, name="ep")
            warm_ps = warm_pool.tile([P, WARM_N], f32)

            # constants via gpsimd (released early in the preamble)
            nc.gpsimd.memset(warm_src[:], 0.0)
            nc.gpsimd.memset(ones_t[:], 1.0)

            # PE warmup: dummy matmuls keep the HAM activity window busy
            # while the first input chunks land.
            for _ in range(N_WARM):
                nc.tensor.matmul(
                    warm_ps, lhsT=warm_src[:, :P], rhs=warm_src[:, :WARM_N],
                    start=True, stop=True,
                )

            # input streams: spins on the sync HWDGE ring, A on the
            # scalar ring, in exact consumption order, small head chunks.
            su3 = su_ext.rearrange("p (k b) -> p k b", b=BH)
            a3 = a_ext.rearrange("p (t q) -> p t q", q=P)
            for ci in range(len(SU_CHUNKS)):
                k0, k1 = SU_CHUNKS[ci]
                nc.sync.dma_start(out=su_t[:, k0:k1], in_=su3[:, k0:k1])
                s0, s1 = A_CHUNKS[ci]
                t0, t1 = _tiles_before(s0), _tiles_before(s1)
                nc.scalar.dma_start(out=a_t[:, t0:t1], in_=a3[:, t0:t1])

            def epilogue(j):
                # m = ps[j] * spin rows of ctile j (position 4j+3), then
                # reduce across partitions via a ones-vector matmul.  The
                # last slot is split into sample halves so the DVE
                # multiply, ones-matmul and PSUM->SBUF copy pipeline.
                halves = (
                    [(0, BH)] if j < 3 else [(0, BH // 2), (BH // 2, BH)]
                )
                for (b0, b1) in halves:
                    nc.vector.scalar_tensor_tensor(
                        m_t[j][:, b0:b1],
                        ps[j][:, b0:b1],
                        1.0,
                        su_t[:, 4 * j + 3, b0:b1],
                        mybir.AluOpType.mult,
                        mybir.AluOpType.mult,
                    )
                    nc.tensor.matmul(
                        ep[:, b0:b1],
                        lhsT=ones_t[:],
                        rhs=m_t[j][:, b0:b1],
                        start=(j == 0),
                        stop=(j == 3),
                    )
                    if j == 3:
                        # PSUM -> SBUF per half (DMA cannot read PSUM)
                        nc.vector.tensor_scalar(
                            red_sb[:, b0:b1], ep[:, b0:b1], 1.0, 0.0,
                            mybir.AluOpType.mult, mybir.AluOpType.add,
                        )

            ti = 0
            for s in range(KT):
                for j in range(4):
                    if S_CAP[j] <= s:
                        continue
                    nc.tensor.matmul(
                        ps[j],
                        lhsT=a_t[:, ti],
                        rhs=su_t[:, s],
                        start=(s == 0),
                        stop=(s == S_CAP[j] - 1),
                    )
                    ti += 1
                for j in range(4):
                    if s == S_CAP[j] - 1:
                        epilogue(j)
            assert ti == NTILE

            nc.sync.dma_start(out=out_ext, in_=red_sb[:])

    nc.compile()
    return nc


def _make_in_maps(state, J):
    bf16 = ml_dtypes.bfloat16
    fp8 = ml_dtypes.float8_e4m3
    state = np.asarray(state, dtype=np.float32)
    J = np.asarray(J, dtype=np.float32)

    # fold the symmetric part into the upper block-triangle
    A = np.triu(J + J.T, 1) + np.diag(np.diag(J))
    A = A.astype(bf16)
    sp8 = np.where(state > 0.5, np.float32(1.0), np.float32(-1.0)).astype(fp8)

    in_maps = []
    placement = []
    for core in range(R * C):
        r, c = divmod(core, C)
        kt_of_pos = []
        for g in range(4):
            grp = [x for x in range(4 * g, 4 * g + 4) if x != 4 * g + r]
            kt_of_pos += grp + [4 * g + r]
        ctile = [4 * j + r for j in range(4)]

        # su: [P, KT, BH]; partition p holds, for position k, the
        # samples of spin row (kt_of_pos[k]*P + p)
        sm = sp8[c * BH:(c + 1) * BH]               # [BH, N]
        su = np.empty((P, KT, BH), dtype=fp8)
        for pos in range(KT):
            kt = kt_of_pos[pos]
            su[:, pos, :] = sm[:, kt * P:(kt + 1) * P].T
        # A tiles in consumption order; zero when above the triangle
        at = np.zeros((P, NTILE, P), dtype=bf16)
        for idx, (s, j) in enumerate(ORDER):
            kt, cj = kt_of_pos[s], ctile[j]
            if kt <= cj:
                at[:, idx, :] = A[kt * P:(kt + 1) * P, cj * P:(cj + 1) * P]
        in_maps.append({
            "su": np.ascontiguousarray(su.reshape(P, KT * BH)),
            "a": np.ascontiguousarray(at.reshape(P, NTILE * P)),
        })
        placement.append((r, c))
    return in_maps, placement


def kernel(state, J):
    from concourse.bass_utils import run_bass_kernel_spmd

    if "nc" not in _cache:
        _cache["nc"] = _build_program()
    nc = _cache["nc"]

    in_maps, placement = _make_in_maps(state, J)
    res = run_bass_kernel_spmd(nc, in_maps, list(range(R * C)))

    out = np.zeros(B, dtype=np.float32)
    for core, (r, c) in enumerate(placement):
        out[c * BH:(c + 1) * BH] += res.results[core]["part"].reshape(BH)
    return out
